# revision 12
# baseline (speedup 1.0000x reference)
"""Multi-head attention TRN2 Bass kernel, 8-way sharded (batch x head-group).

Problem: B=4, S=1536, D=1536, H=8, dk=64, dv=192 (dense_transformer).
Core c handles batch b=c//2 and head group g=c%2 (4 heads, 256 q/k cols,
768 v cols). Inputs are pre-rounded to fp32r (e8m11) on the host; all
matmuls run as float32r (1 PE cycle/row at moving free dim >= 256), the
attention AV matmul runs in bf16 (E=exp(scores) and V quantized to bf16).

Dataflow per core:
  QT/KT = W.T @ x  -> [128 (m within 128-chunk), 2 (head pair), S] in SBUF
  V'    = x @ Wv   -> [128 (s within chunk), 12 (s chunk), 4 (head), 193]
          with column 192 = 1.0: the AV matmul then accumulates the softmax
          denominator (sum of exp) into PSUM column 192 for free.
  scores^T[j, i] = K^T Q per head pair, both heads packed into the
          128-partition contraction dim (dk=64 each) via tile_position row
          groups; exp runs on ACT over the pair's two PSUM banks at once
          with the 1/sqrt(dk) folded into the activation scale.
  out[i, e] = (E @ V') / rowsum, normalized per-partition with a DVE
          reciprocal + tensor_scalar multiply; DMA straight to DRAM.
Phase 2 is software-pipelined (AV of block n emitted after block n+1's
scores) so the PE never waits on the ACT exp stream. Input DMAs ride two
HWDGE rails (qSP: xT, qACT: weights).
"""

import json
from contextlib import ExitStack

import numpy as np

import concourse.bass as bass
import concourse.mybir as mybir
from concourse import tile
from concourse.bass_utils import run_bass_kernel_spmd

FP32R = mybir.dt.float32r
F32 = mybir.dt.float32
BF16 = mybir.dt.bfloat16
AF = mybir.ActivationFunctionType

B = 4
S = 1536
D = 1536
ND = 12  # d chunks of 128
NS = 12  # s chunks of 128
NIB = 3  # i blocks of 512
DV = 192
AV_BF16 = True
IN_DT = mybir.dt.bfloat16  # x and W inputs quantize to bf16 on the host

# Timing-ablation hook (used by abl.py only; None for real runs). Shrinks one
# phase's work while keeping the dependency graph intact, to attribute HW
# time: 'scores' | 'exp' | 'noav' | 'noproj'.
ABLATE = None


# ---------------------------------------------------------------------------
# Workaround: walrus in this container rejects >1 semaphore wait per
# instruction ("Too many sync wait commands"). Splitting the extra waits onto
# preceding same-engine NoOps is semantically identical (engines execute
# their queue in order).
def _split_multi_waits(bir_json: bytes) -> bytes:
    bir = json.loads(bir_json)
    changed = False
    for f in bir.get("functions", []):
        for bb in f.get("blocks", []):
            new_insts = []
            for inst in bb.get("instructions", []):
                si = inst.get("sync_info")
                waits = (si or {}).get("on_wait") or []
                if len(waits) > 1:
                    for k, w in enumerate(waits[:-1]):
                        new_insts.append({
                            "debug": inst.get("debug", 0),
                            "engine": inst["engine"],
                            "ins": [],
                            "name": f"{inst['name']}_wsplit{k}",
                            "opcode": "NoOp",
                            "outs": [],
                            "sync_info": {"on_update": [], "on_wait": [w]},
                        })
                    si["on_wait"] = [waits[-1]]
                    changed = True
                new_insts.append(inst)
            bb["instructions"] = new_insts
    return json.dumps(bir).encode() if changed else bir_json


def _install_waitsplit():
    import concourse.bass_utils as bass_utils
    import concourse.bass2jax as bass2jax

    orig = bass_utils.compile_bir_kernel
    if getattr(orig, "_waitsplit_wrapped", False):
        return

    def patched(bir_json, tmpdir, neff_name="file.neff"):
        return orig(_split_multi_waits(bir_json), tmpdir, neff_name)

    patched._waitsplit_wrapped = True
    bass_utils.compile_bir_kernel = patched
    bass2jax.compile_bir_kernel = patched


# ---------------------------------------------------------------------------
def round_fp32r(x: np.ndarray) -> np.ndarray:
    """Round fp32 to e8m11 (fp32r) with round-to-nearest-even on raw bits."""
    b = np.ascontiguousarray(x, dtype=np.float32).view(np.uint32).astype(np.uint64)
    b = b + 0x7FF + ((b >> 12) & 1)
    b = (b & 0xFFFFF000).astype(np.uint32)
    return b.view(np.float32)


def build_kernel(repeat: int = 1, av_bf16: bool = AV_BF16):
    # av_bf16: store E (exp scores) and V' in bf16 -> AV matmul runs at
    # 1 cyc/row at any free dim, so no 256-pad (N=193) and FWL weight loads.
    e_dt = BF16 if av_bf16 else FP32R
    dvp = (DV + 1) if av_bf16 else 256
    nc = bass.Bass(
        trn_type="TRN2", target_bir_lowering=False, debug=False, num_devices=8
    )
    xT = nc.dram_tensor("xT", [D, S], IN_DT, kind="ExternalInput")
    wq = nc.dram_tensor("wq", [D, 256], IN_DT, kind="ExternalInput")
    wk = nc.dram_tensor("wk", [D, 256], IN_DT, kind="ExternalInput")
    wv = nc.dram_tensor("wv", [D, 768], IN_DT, kind="ExternalInput")
    vpad = nc.dram_tensor("vpad", [128, NS * 4 * (dvp - DV)], e_dt,
                          kind="ExternalInput")
    # head-major output: each [128,192] store is a fully contiguous 96KB
    # block instead of 768B runs at 3072B stride; host gather reshuffles
    out = nc.dram_tensor("out", [4, S, DV], F32, kind="ExternalOutput")

    # [p(d within chunk), c(d chunk), *] views of the D-major dram tensors
    xT_pcs = xT.ap().rearrange("(c p) s -> p c s", p=128)
    wq_pcm = wq.ap().rearrange("(c p) m -> p c m", p=128)
    wk_pcm = wk.ap().rearrange("(c p) m -> p c m", p=128)
    wv_pce = wv.ap().rearrange("(c p) e -> p c e", p=128)
    out_ap = out.ap()

    with tile.TileContext(nc) as tc:
        for _rep in range(repeat):
            _emit_body(nc, tc, xT_pcs, wq_pcm, wk_pcm, wv_pce, vpad, out_ap,
                       e_dt, dvp)
    return nc


def _emit_body(nc, tc, xT_pcs, wq_pcm, wk_pcm, wv_pce, vpad, out_ap, e_dt, dvp):
    with ExitStack() as ctx:
        persist = ctx.enter_context(tc.tile_pool(name="persist", bufs=1))
        # disjoint PSUM pools for the whole body: no cross-phase bank reuse,
        # so later phases never wait on earlier phases' PSUM readers.
        # proj(2) + scores(2x2) + av(2) = 8 banks.
        # projection chains (phase 1) and AV chains (phase 2) share one
        # 4-slot pool (same tag -> same banks): 4 + scores 2x2 = 8 banks,
        # giving both phases twice the chain-level double-buffering
        p_mix = ctx.enter_context(tc.tile_pool(name="p_mix", bufs=4, space="PSUM"))
        p_proj = p_av = p_mix
        p_sc = ctx.enter_context(tc.tile_pool(name="p_sc", bufs=2, space="PSUM"))
        mp = ctx.enter_context(tc.tile_pool(name="mp", bufs=4))

        # Q/K live in bf16: the scores matmul contracts over 128 rows (two
        # 64-row head halves), so K is stored twice with complementary
        # zero-padded halves ([K_A;0] and [0;K_B]) — a 128-row-weight matmul
        # streams at ~0.52 ns/row on HW vs ~0.85 ns/row for 64-row weights.
        qt = persist.tile([128, 2, S], BF16)
        ktp = persist.tile([128, 2, 2, S], BF16)  # [p, variant A/B, pair, s]
        vp = persist.tile([128, NS, 4, dvp], e_dt)
        # zero the pad halves on the idle Pool engine during the input DMA
        nc.gpsimd.memset(ktp[64:128, 0, :, :], 0.0)
        nc.gpsimd.memset(ktp[0:64, 1, :, :], 0.0)

        # ones column (softmax denominator) + zero pad, from DRAM.
        # Input DMAs ride two independent HWDGE rails: qSP (nc.sync) carries
        # xT, qACT (nc.scalar) carries the weights; first-needed first.
        nc.scalar.dma_start(
            vp[:, :, :, DV:dvp],
            vpad.ap().rearrange("p (c h e) -> p c h e", c=NS, h=4),
        )

        with ExitStack() as s1:
            xa = s1.enter_context(tc.tile_pool(name="xa", bufs=1))
            # wv prefetched on the ACT rail during phase 1a; its pool sits
            # below wqk on the stack so the prefetch isn't gated on wqk reuse
            wvp = s1.enter_context(tc.tile_pool(name="wvp", bufs=1))
            wv_sb = wvp.tile([128, ND, 768], IN_DT)

            # ---- Phase 1a: QT = Wq.T @ x, KT = Wk.T @ x (m on partitions)
            with ExitStack() as s1a:
                wqk = s1a.enter_context(tc.tile_pool(name="wqk", bufs=1))
                wq_sb = wqk.tile([128, ND, 256], IN_DT)
                wk_sb = wqk.tile([128, ND, 256], IN_DT)
                for dc2 in range(0, ND, 2):
                    nc.scalar.dma_start(
                        wq_sb[:, dc2 : dc2 + 2, :], wq_pcm[:, dc2 : dc2 + 2, :]
                    )
                for dc4 in range(0, ND, 4):
                    nc.scalar.dma_start(
                        wk_sb[:, dc4 : dc4 + 4, :], wk_pcm[:, dc4 : dc4 + 4, :]
                    )
                xtile = xa.tile([128, ND, S], IN_DT)
                # split across s-blocks and d-chunks so HWDGE queues overlap;
                # first block per-chunk so the first chain starts sooner
                for dc in range(ND):
                    nc.sync.dma_start(
                        xtile[:, dc, 0:512], xT_pcs[:, dc, 0:512]
                    )
                for dc2 in range(0, ND, 2):
                    nc.sync.dma_start(
                        xtile[:, dc2 : dc2 + 2, 512:S],
                        xT_pcs[:, dc2 : dc2 + 2, 512:S],
                    )
                for dc2 in range(0, ND, 2):
                    nc.scalar.dma_start(
                        wv_sb[:, dc2 : dc2 + 2, :], wv_pce[:, dc2 : dc2 + 2, :]
                    )
                pnd = 3 if ABLATE == "noproj" else ND
                for ib in range(NIB):
                    for w_sb, is_q in ((wq_sb, True), (wk_sb, False)):
                        for m2 in range(2):
                            ps = p_proj.tile([128, 512], F32, tag="pmix")
                            for dc in range(pnd):
                                nc.tensor.matmul(
                                    ps[:],
                                    w_sb[:, dc, m2 * 128 : (m2 + 1) * 128],
                                    xtile[:, dc, ib * 512 : (ib + 1) * 512],
                                    start=(dc == 0),
                                    stop=(dc == pnd - 1),
                                )
                            blk = slice(ib * 512, (ib + 1) * 512)
                            if is_q:
                                nc.vector.tensor_copy(qt[:, m2, blk], ps[:])
                            else:
                                nc.vector.tensor_copy(
                                    ktp[0:64, 0, m2, blk], ps[0:64, :]
                                )
                                nc.vector.tensor_copy(
                                    ktp[64:128, 1, m2, blk], ps[64:128, :]
                                )

            # ---- Phase 2 setup: the first two blocks' scores are emitted
            # before the V projection so their ACT exp stream hides entirely
            # under phase 1b's PE work; the rest runs as a lag-1 software
            # pipeline (AV of block n after block n+1's scores) so the PE
            # never waits on the exp stream.
            ep = s1.enter_context(tc.tile_pool(name="ep", bufs=2))

            def emit_scores(pair, ib):
                i0 = ib * 512
                # E holds exp(scores^T/8) for both heads of the pair:
                # head A in [:, jc, 0:512], head B in [:, jc, 512:1024]
                e_sb = ep.tile([128, NS, 1024], e_dt, tag="e")
                mv = 64 if ABLATE == "scores" else 512
                for jc in range(NS):
                    j0 = jc * 128
                    pss = p_sc.tile([128, 1024], F32, tag="pss")
                    nc.tensor.matmul(
                        pss[:, 0:mv],
                        ktp[:, 0, pair, j0 : j0 + 128],
                        qt[:, pair, i0 : i0 + mv],
                        start=True,
                        stop=True,
                    )
                    nc.tensor.matmul(
                        pss[:, 512 : 512 + mv],
                        ktp[:, 1, pair, j0 : j0 + 128],
                        qt[:, pair, i0 : i0 + mv],
                        start=True,
                        stop=True,
                    )
                    if ABLATE == "exp":
                        nc.scalar.activation(
                            e_sb[:, jc, 0:64], pss[:, 0:64], AF.Exp, scale=0.125
                        )
                    else:
                        nc.scalar.activation(
                            e_sb[:, jc, :], pss[:], AF.Exp, scale=0.125
                        )
                return e_sb

            def emit_av(pair, ib, e_sb):
                i0 = ib * 512
                njc = 1 if ABLATE == "noav" else NS
                for hh in range(2):
                    h = pair * 2 + hh
                    for isub in range(4):
                        pav = p_av.tile([128, dvp], F32, tag="pmix")
                        for jc in range(njc):
                            nc.tensor.matmul(
                                pav[:],
                                e_sb[
                                    :,
                                    jc,
                                    hh * 512 + isub * 128 : hh * 512
                                    + (isub + 1) * 128,
                                ],
                                vp[:, jc, h, :],
                                start=(jc == 0),
                                stop=(jc == njc - 1),
                            )
                        rec = mp.tile([128, 1], F32, tag="rec")
                        nc.vector.reciprocal(rec[:], pav[:, DV : DV + 1])
                        ot = mp.tile([128, DV], F32, tag="ot")
                        nc.vector.tensor_scalar_mul(ot[:], pav[:, 0:DV], rec[:])
                        r0 = i0 + isub * 128
                        nc.scalar.dma_start(out_ap[h, r0 : r0 + 128, :], ot[:])

            blocks = [(pair, ib) for pair in range(2) for ib in range(NIB)]
            pending = []
            for pair, ib in blocks[:2]:
                pending.append((pair, ib, emit_scores(pair, ib)))

            # ---- Phase 1b: V = x @ Wv (natural layout: s on partitions)
            vnd = 3 if ABLATE == "noproj" else ND
            for sc in range(NS):
                c0 = sc * 128
                for e2 in range(2):
                    ps = p_proj.tile([128, 384], F32, tag="pmix")
                    for dc in range(vnd):
                        nc.tensor.matmul(
                            ps[:],
                            xtile[:, dc, c0 : c0 + 128],
                            wv_sb[:, dc, e2 * 384 : (e2 + 1) * 384],
                            start=(dc == 0),
                            stop=(dc == vnd - 1),
                        )
                    nc.vector.tensor_copy(vp[:, sc, 2 * e2, 0:DV], ps[:, 0:DV])
                    nc.vector.tensor_copy(
                        vp[:, sc, 2 * e2 + 1, 0:DV], ps[:, DV : 2 * DV]
                    )

            # ---- Phase 2 tail
            for pair, ib in blocks[2:]:
                emit_av(*pending.pop(0))
                pending.append((pair, ib, emit_scores(pair, ib)))
            for blk in pending:
                emit_av(*blk)


def make_vpad() -> np.ndarray:
    import ml_dtypes

    dvp = (DV + 1) if AV_BF16 else 256
    dt = ml_dtypes.bfloat16 if AV_BF16 else np.float32
    pad = np.zeros((128, NS, 4, dvp - DV), dt)
    pad[:, :, :, 0] = 1.0
    return pad.reshape(128, -1)


def shard_inputs(inputs, Wq, Wk, Wv):
    vpad = make_vpad()
    import ml_dtypes

    bf = ml_dtypes.bfloat16
    in_maps = []
    for c in range(8):
        b, g = c // 2, c % 2
        in_maps.append(
            {
                "xT": np.asarray(inputs[b]).T.astype(bf),
                "wq": np.asarray(Wq[:, g * 256 : (g + 1) * 256]).astype(bf),
                "wk": np.asarray(Wk[:, g * 256 : (g + 1) * 256]).astype(bf),
                "wv": np.asarray(Wv[:, g * 768 : (g + 1) * 768]).astype(bf),
                "vpad": vpad,
            }
        )
    return in_maps


def gather_outputs(results):
    full = np.empty((B, S, 1536), np.float32)
    for c, r in enumerate(results):
        b, g = c // 2, c % 2
        o = r["out"]
        for h in range(4):
            full[b, :, g * 768 + h * DV : g * 768 + (h + 1) * DV] = o[h]
    return full


_cached = {}


def kernel(inputs, Wq, Wk, Wv) -> np.ndarray:
    """Full [4,1536,1536] fp32 MHA forward, computed on 8 NeuronCores."""
    _install_waitsplit()
    inputs = np.asarray(inputs, dtype=np.float32)
    Wq = np.asarray(Wq, dtype=np.float32)
    Wk = np.asarray(Wk, dtype=np.float32)
    Wv = np.asarray(Wv, dtype=np.float32)

    if "nc" not in _cached:
        _cached["nc"] = build_kernel()
    nc = _cached["nc"]
    in_maps = shard_inputs(inputs, Wq, Wk, Wv)

    last_err = None
    for _attempt in range(3):
        try:
            res = run_bass_kernel_spmd(nc, in_maps, core_ids=list(range(8)))
            return gather_outputs(res.results)
        except Exception as e:  # wedged-device retry
            last_err = e
    raise last_err



# revision 13
# speedup vs baseline: 1.0616x; 1.0616x over previous
"""Multi-head attention TRN2 Bass kernel, 8-way sharded (batch x head-group).

Problem: B=4, S=1536, D=1536, H=8, dk=64, dv=192 (dense_transformer).
Core c handles batch b=c//2 and head group g=c%2 (4 heads, 256 q/k cols,
768 v cols). Inputs are pre-rounded to fp32r (e8m11) on the host; all
matmuls run as float32r (1 PE cycle/row at moving free dim >= 256), the
attention AV matmul runs in bf16 (E=exp(scores) and V quantized to bf16).

Dataflow per core:
  QT/KT = W.T @ x  -> [128 (m within 128-chunk), 2 (head pair), S] in SBUF
  V'    = x @ Wv   -> [128 (s within chunk), 12 (s chunk), 4 (head), 193]
          with column 192 = 1.0: the AV matmul then accumulates the softmax
          denominator (sum of exp) into PSUM column 192 for free.
  scores^T[j, i] = K^T Q per head pair, both heads packed into the
          128-partition contraction dim (dk=64 each) via tile_position row
          groups; exp runs on ACT over the pair's two PSUM banks at once
          with the 1/sqrt(dk) folded into the activation scale.
  out[i, e] = (E @ V') / rowsum, normalized per-partition with a DVE
          reciprocal + tensor_scalar multiply; DMA straight to DRAM.
Phase 2 is software-pipelined (AV of block n emitted after block n+1's
scores) so the PE never waits on the ACT exp stream. Input DMAs ride two
HWDGE rails (qSP: xT, qACT: weights).
"""

import json
from contextlib import ExitStack

import numpy as np

import concourse.bass as bass
import concourse.mybir as mybir
from concourse import tile
from concourse.bass_utils import run_bass_kernel_spmd

FP32R = mybir.dt.float32r
F32 = mybir.dt.float32
BF16 = mybir.dt.bfloat16
AF = mybir.ActivationFunctionType

B = 4
S = 1536
D = 1536
ND = 12  # d chunks of 128
NS = 12  # s chunks of 128
NIB = 3  # i blocks of 512
DV = 192
AV_BF16 = True
IN_DT = mybir.dt.bfloat16  # x and W inputs quantize to bf16 on the host

# Timing-ablation hook (used by abl.py only; None for real runs). Shrinks one
# phase's work while keeping the dependency graph intact, to attribute HW
# time: 'scores' | 'exp' | 'noav' | 'noproj'.
ABLATE = None


# ---------------------------------------------------------------------------
# Workaround: walrus in this container rejects >1 semaphore wait per
# instruction ("Too many sync wait commands"). Splitting the extra waits onto
# preceding same-engine NoOps is semantically identical (engines execute
# their queue in order).
def _split_multi_waits(bir_json: bytes) -> bytes:
    bir = json.loads(bir_json)
    changed = False
    for f in bir.get("functions", []):
        for bb in f.get("blocks", []):
            new_insts = []
            for inst in bb.get("instructions", []):
                si = inst.get("sync_info")
                waits = (si or {}).get("on_wait") or []
                if len(waits) > 1:
                    for k, w in enumerate(waits[:-1]):
                        new_insts.append({
                            "debug": inst.get("debug", 0),
                            "engine": inst["engine"],
                            "ins": [],
                            "name": f"{inst['name']}_wsplit{k}",
                            "opcode": "NoOp",
                            "outs": [],
                            "sync_info": {"on_update": [], "on_wait": [w]},
                        })
                    si["on_wait"] = [waits[-1]]
                    changed = True
                new_insts.append(inst)
            bb["instructions"] = new_insts
    return json.dumps(bir).encode() if changed else bir_json


def _install_waitsplit():
    import concourse.bass_utils as bass_utils
    import concourse.bass2jax as bass2jax

    orig = bass_utils.compile_bir_kernel
    if getattr(orig, "_waitsplit_wrapped", False):
        return

    def patched(bir_json, tmpdir, neff_name="file.neff"):
        return orig(_split_multi_waits(bir_json), tmpdir, neff_name)

    patched._waitsplit_wrapped = True
    bass_utils.compile_bir_kernel = patched
    bass2jax.compile_bir_kernel = patched


# ---------------------------------------------------------------------------
def round_fp32r(x: np.ndarray) -> np.ndarray:
    """Round fp32 to e8m11 (fp32r) with round-to-nearest-even on raw bits."""
    b = np.ascontiguousarray(x, dtype=np.float32).view(np.uint32).astype(np.uint64)
    b = b + 0x7FF + ((b >> 12) & 1)
    b = (b & 0xFFFFF000).astype(np.uint32)
    return b.view(np.float32)


def build_kernel(repeat: int = 1, av_bf16: bool = AV_BF16):
    # av_bf16: store E (exp scores) and V' in bf16 -> AV matmul runs at
    # 1 cyc/row at any free dim, so no 256-pad (N=193) and FWL weight loads.
    e_dt = BF16 if av_bf16 else FP32R
    dvp = (DV + 1) if av_bf16 else 256
    nc = bass.Bass(
        trn_type="TRN2", target_bir_lowering=False, debug=False, num_devices=8
    )
    xT = nc.dram_tensor("xT", [D, S], IN_DT, kind="ExternalInput")
    wq = nc.dram_tensor("wq", [D, 256], IN_DT, kind="ExternalInput")
    wk = nc.dram_tensor("wk", [D, 256], IN_DT, kind="ExternalInput")
    wv = nc.dram_tensor("wv", [D, 768], IN_DT, kind="ExternalInput")
    vpad = nc.dram_tensor("vpad", [128, NS * 4 * (dvp - DV)], e_dt,
                          kind="ExternalInput")
    # head-major output: each [128,192] store is a fully contiguous 96KB
    # block instead of 768B runs at 3072B stride; host gather reshuffles
    out = nc.dram_tensor("out", [4, S, DV], F32, kind="ExternalOutput")

    # [p(d within chunk), c(d chunk), *] views of the D-major dram tensors
    xT_pcs = xT.ap().rearrange("(c p) s -> p c s", p=128)
    wq_pcm = wq.ap().rearrange("(c p) m -> p c m", p=128)
    wk_pcm = wk.ap().rearrange("(c p) m -> p c m", p=128)
    wv_pce = wv.ap().rearrange("(c p) e -> p c e", p=128)
    out_ap = out.ap()

    with tile.TileContext(nc) as tc:
        for _rep in range(repeat):
            _emit_body(nc, tc, xT_pcs, wq_pcm, wk_pcm, wv_pce, vpad, out_ap,
                       e_dt, dvp)
    return nc


def _emit_body(nc, tc, xT_pcs, wq_pcm, wk_pcm, wv_pce, vpad, out_ap, e_dt, dvp):
    with ExitStack() as ctx:
        persist = ctx.enter_context(tc.tile_pool(name="persist", bufs=1))
        # disjoint PSUM pools for the whole body: no cross-phase bank reuse,
        # so later phases never wait on earlier phases' PSUM readers.
        # proj(2) + scores(2x2) + av(2) = 8 banks.
        # projection chains (phase 1) and AV chains (phase 2) share one
        # 4-slot pool (same tag -> same banks): 4 + scores 2x2 = 8 banks,
        # giving both phases twice the chain-level double-buffering
        p_mix = ctx.enter_context(tc.tile_pool(name="p_mix", bufs=4, space="PSUM"))
        p_proj = p_av = p_mix
        p_sc = ctx.enter_context(tc.tile_pool(name="p_sc", bufs=2, space="PSUM"))
        mp = ctx.enter_context(tc.tile_pool(name="mp", bufs=4))

        # Q/K live in bf16: the scores matmul contracts over 128 rows (two
        # 64-row head halves), so K is stored twice with complementary
        # zero-padded halves ([K_A;0] and [0;K_B]) — a 128-row-weight matmul
        # streams at ~0.52 ns/row on HW vs ~0.85 ns/row for 64-row weights.
        qt = persist.tile([128, 2, S], BF16)
        ktp = persist.tile([128, 2, 2, S], BF16)  # [p, variant A/B, pair, s]
        vp = persist.tile([128, NS, 4, dvp], e_dt)
        # zero the pad halves on the idle Pool engine during the input DMA
        nc.gpsimd.memset(ktp[64:128, 0, :, :], 0.0)
        nc.gpsimd.memset(ktp[0:64, 1, :, :], 0.0)

        # ones column (softmax denominator) + zero pad, from DRAM.
        # Input DMAs ride two independent HWDGE rails: qSP (nc.sync) carries
        # xT, qACT (nc.scalar) carries the weights; first-needed first.
        nc.scalar.dma_start(
            vp[:, :, :, DV:dvp],
            vpad.ap().rearrange("p (c h e) -> p c h e", c=NS, h=4),
        )

        with ExitStack() as s1:
            xa = s1.enter_context(tc.tile_pool(name="xa", bufs=1))
            # wv prefetched on the ACT rail during phase 1a; its pool sits
            # below wqk on the stack so the prefetch isn't gated on wqk reuse
            wvp = s1.enter_context(tc.tile_pool(name="wvp", bufs=1))
            wv_sb = wvp.tile([128, ND, 768], IN_DT)

            # ---- Phase 1a: QT = Wq.T @ x, KT = Wk.T @ x (m on partitions)
            with ExitStack() as s1a:
                wqk = s1a.enter_context(tc.tile_pool(name="wqk", bufs=1))
                wq_sb = wqk.tile([128, ND, 256], IN_DT)
                wk_sb = wqk.tile([128, ND, 256], IN_DT)
                for dc2 in range(0, ND, 2):
                    nc.scalar.dma_start(
                        wq_sb[:, dc2 : dc2 + 2, :], wq_pcm[:, dc2 : dc2 + 2, :]
                    )
                for dc4 in range(0, ND, 4):
                    nc.scalar.dma_start(
                        wk_sb[:, dc4 : dc4 + 4, :], wk_pcm[:, dc4 : dc4 + 4, :]
                    )
                xtile = xa.tile([128, ND, S], IN_DT)
                # split across s-blocks and d-chunks so HWDGE queues overlap;
                # first block per-chunk so the first chain starts sooner
                for dc in range(ND):
                    nc.sync.dma_start(
                        xtile[:, dc, 0:512], xT_pcs[:, dc, 0:512]
                    )
                for dc2 in range(0, ND, 2):
                    nc.sync.dma_start(
                        xtile[:, dc2 : dc2 + 2, 512:S],
                        xT_pcs[:, dc2 : dc2 + 2, 512:S],
                    )
                for dc2 in range(0, ND, 2):
                    nc.scalar.dma_start(
                        wv_sb[:, dc2 : dc2 + 2, :], wv_pce[:, dc2 : dc2 + 2, :]
                    )
                pnd = 3 if ABLATE == "noproj" else ND
                for ib in range(NIB):
                    for w_sb, is_q in ((wq_sb, True), (wk_sb, False)):
                        for m2 in range(2):
                            ps = p_proj.tile([128, 512], F32, tag="pmix")
                            for dc in range(pnd):
                                nc.tensor.matmul(
                                    ps[:],
                                    w_sb[:, dc, m2 * 128 : (m2 + 1) * 128],
                                    xtile[:, dc, ib * 512 : (ib + 1) * 512],
                                    start=(dc == 0),
                                    stop=(dc == pnd - 1),
                                )
                            blk = slice(ib * 512, (ib + 1) * 512)
                            if is_q:
                                nc.vector.tensor_copy(qt[:, m2, blk], ps[:])
                            else:
                                nc.vector.tensor_copy(
                                    ktp[0:64, 0, m2, blk], ps[0:64, :]
                                )
                                nc.vector.tensor_copy(
                                    ktp[64:128, 1, m2, blk], ps[64:128, :]
                                )

            # ---- Phase 2 setup: the first two blocks' scores are emitted
            # before the V projection so their ACT exp stream hides entirely
            # under phase 1b's PE work; the rest runs as a lag-1 software
            # pipeline (AV of block n after block n+1's scores) so the PE
            # never waits on the exp stream.
            ep = s1.enter_context(tc.tile_pool(name="ep", bufs=2))

            def emit_scores(pair, ib):
                i0 = ib * 512
                # E holds exp(scores^T/8) for both heads of the pair:
                # head A in [:, jc, 0:512], head B in [:, jc, 512:1024]
                e_sb = ep.tile([128, NS, 1024], e_dt, tag="e")
                mv = 64 if ABLATE == "scores" else 512
                for jc in range(NS):
                    j0 = jc * 128
                    pss = p_sc.tile([128, 1024], F32, tag="pss")
                    nc.tensor.matmul(
                        pss[:, 0:mv],
                        ktp[:, 0, pair, j0 : j0 + 128],
                        qt[:, pair, i0 : i0 + mv],
                        start=True,
                        stop=True,
                    )
                    nc.tensor.matmul(
                        pss[:, 512 : 512 + mv],
                        ktp[:, 1, pair, j0 : j0 + 128],
                        qt[:, pair, i0 : i0 + mv],
                        start=True,
                        stop=True,
                    )
                    if ABLATE == "exp":
                        nc.scalar.activation(
                            e_sb[:, jc, 0:64], pss[:, 0:64], AF.Exp, scale=0.125
                        )
                    else:
                        nc.scalar.activation(
                            e_sb[:, jc, :], pss[:], AF.Exp, scale=0.125
                        )
                return e_sb

            def emit_av(pair, ib, e_sb):
                i0 = ib * 512
                njc = 1 if ABLATE == "noav" else NS
                for hh in range(2):
                    h = pair * 2 + hh
                    for isub in range(4):
                        pav = p_av.tile([128, dvp], F32, tag="pmix")
                        for jc in range(njc):
                            nc.tensor.matmul(
                                pav[:],
                                e_sb[
                                    :,
                                    jc,
                                    hh * 512 + isub * 128 : hh * 512
                                    + (isub + 1) * 128,
                                ],
                                vp[:, jc, h, :],
                                start=(jc == 0),
                                stop=(jc == njc - 1),
                            )
                        rec = mp.tile([128, 1], F32, tag="rec")
                        nc.vector.reciprocal(rec[:], pav[:, DV : DV + 1])
                        ot = mp.tile([128, DV], F32, tag="ot")
                        nc.vector.tensor_scalar_mul(ot[:], pav[:, 0:DV], rec[:])
                        r0 = i0 + isub * 128
                        nc.sync.dma_start(out_ap[h, r0 : r0 + 128, :], ot[:])

            blocks = [(pair, ib) for pair in range(2) for ib in range(NIB)]
            pending = []
            for pair, ib in blocks[:2]:
                pending.append((pair, ib, emit_scores(pair, ib)))

            # ---- Phase 1b: V = x @ Wv (natural layout: s on partitions)
            vnd = 3 if ABLATE == "noproj" else ND
            for sc in range(NS):
                c0 = sc * 128
                for e2 in range(2):
                    ps = p_proj.tile([128, 384], F32, tag="pmix")
                    for dc in range(vnd):
                        nc.tensor.matmul(
                            ps[:],
                            xtile[:, dc, c0 : c0 + 128],
                            wv_sb[:, dc, e2 * 384 : (e2 + 1) * 384],
                            start=(dc == 0),
                            stop=(dc == vnd - 1),
                        )
                    nc.vector.tensor_copy(vp[:, sc, 2 * e2, 0:DV], ps[:, 0:DV])
                    nc.vector.tensor_copy(
                        vp[:, sc, 2 * e2 + 1, 0:DV], ps[:, DV : 2 * DV]
                    )

            # ---- Phase 2 tail
            for pair, ib in blocks[2:]:
                emit_av(*pending.pop(0))
                pending.append((pair, ib, emit_scores(pair, ib)))
            for blk in pending:
                emit_av(*blk)


def make_vpad() -> np.ndarray:
    import ml_dtypes

    dvp = (DV + 1) if AV_BF16 else 256
    dt = ml_dtypes.bfloat16 if AV_BF16 else np.float32
    pad = np.zeros((128, NS, 4, dvp - DV), dt)
    pad[:, :, :, 0] = 1.0
    return pad.reshape(128, -1)


def shard_inputs(inputs, Wq, Wk, Wv):
    vpad = make_vpad()
    import ml_dtypes

    bf = ml_dtypes.bfloat16
    in_maps = []
    for c in range(8):
        b, g = c // 2, c % 2
        in_maps.append(
            {
                "xT": np.asarray(inputs[b]).T.astype(bf),
                "wq": np.asarray(Wq[:, g * 256 : (g + 1) * 256]).astype(bf),
                "wk": np.asarray(Wk[:, g * 256 : (g + 1) * 256]).astype(bf),
                "wv": np.asarray(Wv[:, g * 768 : (g + 1) * 768]).astype(bf),
                "vpad": vpad,
            }
        )
    return in_maps


def gather_outputs(results):
    full = np.empty((B, S, 1536), np.float32)
    for c, r in enumerate(results):
        b, g = c // 2, c % 2
        o = r["out"]
        for h in range(4):
            full[b, :, g * 768 + h * DV : g * 768 + (h + 1) * DV] = o[h]
    return full


_cached = {}


def kernel(inputs, Wq, Wk, Wv) -> np.ndarray:
    """Full [4,1536,1536] fp32 MHA forward, computed on 8 NeuronCores."""
    _install_waitsplit()
    inputs = np.asarray(inputs, dtype=np.float32)
    Wq = np.asarray(Wq, dtype=np.float32)
    Wk = np.asarray(Wk, dtype=np.float32)
    Wv = np.asarray(Wv, dtype=np.float32)

    if "nc" not in _cached:
        _cached["nc"] = build_kernel()
    nc = _cached["nc"]
    in_maps = shard_inputs(inputs, Wq, Wk, Wv)

    last_err = None
    for _attempt in range(3):
        try:
            res = run_bass_kernel_spmd(nc, in_maps, core_ids=list(range(8)))
            return gather_outputs(res.results)
        except Exception as e:  # wedged-device retry
            last_err = e
    raise last_err



# revision 14
# speedup vs baseline: 1.0959x; 1.0323x over previous
"""Multi-head attention TRN2 Bass kernel, 8-way sharded (batch x head-group).

Problem: B=4, S=1536, D=1536, H=8, dk=64, dv=192 (dense_transformer).
Core c handles batch b=c//2 and head group g=c%2 (4 heads, 256 q/k cols,
768 v cols). Inputs are pre-rounded to fp32r (e8m11) on the host; all
matmuls run as float32r (1 PE cycle/row at moving free dim >= 256), the
attention AV matmul runs in bf16 (E=exp(scores) and V quantized to bf16).

Dataflow per core:
  QT/KT = W.T @ x  -> [128 (m within 128-chunk), 2 (head pair), S] in SBUF
  V'    = x @ Wv   -> [128 (s within chunk), 12 (s chunk), 4 (head), 193]
          with column 192 = 1.0: the AV matmul then accumulates the softmax
          denominator (sum of exp) into PSUM column 192 for free.
  scores^T[j, i] = K^T Q per head pair, both heads packed into the
          128-partition contraction dim (dk=64 each) via tile_position row
          groups; exp runs on ACT over the pair's two PSUM banks at once
          with the 1/sqrt(dk) folded into the activation scale.
  out[i, e] = (E @ V') / rowsum, normalized per-partition with a DVE
          reciprocal + tensor_scalar multiply; DMA straight to DRAM.
Phase 2 is software-pipelined (AV of block n emitted after block n+1's
scores) so the PE never waits on the ACT exp stream. Input DMAs ride two
HWDGE rails (qSP: xT, qACT: weights).
"""

import json
from contextlib import ExitStack

import numpy as np

import concourse.bass as bass
import concourse.mybir as mybir
from concourse import tile
from concourse.bass_utils import run_bass_kernel_spmd

FP32R = mybir.dt.float32r
F32 = mybir.dt.float32
BF16 = mybir.dt.bfloat16
AF = mybir.ActivationFunctionType

B = 4
S = 1536
D = 1536
ND = 12  # d chunks of 128
NS = 12  # s chunks of 128
NIB = 3  # i blocks of 512
DV = 192
AV_BF16 = True
IN_DT = mybir.dt.bfloat16  # x and W inputs quantize to bf16 on the host

# Timing-ablation hook (used by abl.py only; None for real runs). Shrinks one
# phase's work while keeping the dependency graph intact, to attribute HW
# time: 'scores' | 'exp' | 'noav' | 'noproj'.
ABLATE = None


# ---------------------------------------------------------------------------
# Workaround: walrus in this container rejects >1 semaphore wait per
# instruction ("Too many sync wait commands"). Splitting the extra waits onto
# preceding same-engine NoOps is semantically identical (engines execute
# their queue in order).
def _split_multi_waits(bir_json: bytes) -> bytes:
    bir = json.loads(bir_json)
    changed = False
    for f in bir.get("functions", []):
        for bb in f.get("blocks", []):
            new_insts = []
            for inst in bb.get("instructions", []):
                si = inst.get("sync_info")
                waits = (si or {}).get("on_wait") or []
                if len(waits) > 1:
                    for k, w in enumerate(waits[:-1]):
                        new_insts.append({
                            "debug": inst.get("debug", 0),
                            "engine": inst["engine"],
                            "ins": [],
                            "name": f"{inst['name']}_wsplit{k}",
                            "opcode": "NoOp",
                            "outs": [],
                            "sync_info": {"on_update": [], "on_wait": [w]},
                        })
                    si["on_wait"] = [waits[-1]]
                    changed = True
                new_insts.append(inst)
            bb["instructions"] = new_insts
    return json.dumps(bir).encode() if changed else bir_json


def _install_waitsplit():
    import concourse.bass_utils as bass_utils
    import concourse.bass2jax as bass2jax

    orig = bass_utils.compile_bir_kernel
    if getattr(orig, "_waitsplit_wrapped", False):
        return

    def patched(bir_json, tmpdir, neff_name="file.neff"):
        return orig(_split_multi_waits(bir_json), tmpdir, neff_name)

    patched._waitsplit_wrapped = True
    bass_utils.compile_bir_kernel = patched
    bass2jax.compile_bir_kernel = patched


# ---------------------------------------------------------------------------
def round_fp32r(x: np.ndarray) -> np.ndarray:
    """Round fp32 to e8m11 (fp32r) with round-to-nearest-even on raw bits."""
    b = np.ascontiguousarray(x, dtype=np.float32).view(np.uint32).astype(np.uint64)
    b = b + 0x7FF + ((b >> 12) & 1)
    b = (b & 0xFFFFF000).astype(np.uint32)
    return b.view(np.float32)


def build_kernel(repeat: int = 1, av_bf16: bool = AV_BF16):
    # av_bf16: store E (exp scores) and V' in bf16 -> AV matmul runs at
    # 1 cyc/row at any free dim, so no 256-pad (N=193) and FWL weight loads.
    e_dt = BF16 if av_bf16 else FP32R
    dvp = (DV + 1) if av_bf16 else 256
    nc = bass.Bass(
        trn_type="TRN2", target_bir_lowering=False, debug=False, num_devices=8
    )
    xT = nc.dram_tensor("xT", [D, S], IN_DT, kind="ExternalInput")
    wq = nc.dram_tensor("wq", [D, 256], IN_DT, kind="ExternalInput")
    wk = nc.dram_tensor("wk", [D, 256], IN_DT, kind="ExternalInput")
    wv = nc.dram_tensor("wv", [D, 768], IN_DT, kind="ExternalInput")
    vpad = nc.dram_tensor("vpad", [128, NS * 4 * (dvp - DV)], e_dt,
                          kind="ExternalInput")
    # head-major output: each [128,192] store is a fully contiguous 96KB
    # block instead of 768B runs at 3072B stride; host gather reshuffles
    out = nc.dram_tensor("out", [4, S, DV], F32, kind="ExternalOutput")

    # [p(d within chunk), c(d chunk), *] views of the D-major dram tensors
    xT_pcs = xT.ap().rearrange("(c p) s -> p c s", p=128)
    wq_pcm = wq.ap().rearrange("(c p) m -> p c m", p=128)
    wk_pcm = wk.ap().rearrange("(c p) m -> p c m", p=128)
    wv_pce = wv.ap().rearrange("(c p) e -> p c e", p=128)
    out_ap = out.ap()

    with tile.TileContext(nc) as tc:
        for _rep in range(repeat):
            _emit_body(nc, tc, xT_pcs, wq_pcm, wk_pcm, wv_pce, vpad, out_ap,
                       e_dt, dvp)
    return nc


def _emit_body(nc, tc, xT_pcs, wq_pcm, wk_pcm, wv_pce, vpad, out_ap, e_dt, dvp):
    with ExitStack() as ctx:
        persist = ctx.enter_context(tc.tile_pool(name="persist", bufs=1))
        # disjoint PSUM pools for the whole body: no cross-phase bank reuse,
        # so later phases never wait on earlier phases' PSUM readers.
        # proj(2) + scores(2x2) + av(2) = 8 banks.
        # projection chains (phase 1) and AV chains (phase 2) share one
        # 4-slot pool (same tag -> same banks): 4 + scores 2x2 = 8 banks,
        # giving both phases twice the chain-level double-buffering
        p_mix = ctx.enter_context(tc.tile_pool(name="p_mix", bufs=4, space="PSUM"))
        p_proj = p_av = p_mix
        p_sc = ctx.enter_context(tc.tile_pool(name="p_sc", bufs=2, space="PSUM"))
        mp = ctx.enter_context(tc.tile_pool(name="mp", bufs=4))

        # Q/K live in bf16: the scores matmul contracts over 128 rows (two
        # 64-row head halves), so K is stored twice with complementary
        # zero-padded halves ([K_A;0] and [0;K_B]) — a 128-row-weight matmul
        # streams at ~0.52 ns/row on HW vs ~0.85 ns/row for 64-row weights.
        qt = persist.tile([128, 2, S], BF16)
        ktp = persist.tile([128, 2, 2, S], BF16)  # [p, variant A/B, pair, s]
        vp = persist.tile([128, NS, 4, dvp], e_dt)
        # zero the pad halves on the idle Pool engine during the input DMA
        nc.gpsimd.memset(ktp[64:128, 0, :, :], 0.0)
        nc.gpsimd.memset(ktp[0:64, 1, :, :], 0.0)

        # ones column (softmax denominator) + zero pad, from DRAM.
        # Input DMAs ride two independent HWDGE rails: qSP (nc.sync) carries
        # xT, qACT (nc.scalar) carries the weights; first-needed first.
        nc.scalar.dma_start(
            vp[:, :, :, DV:dvp],
            vpad.ap().rearrange("p (c h e) -> p c h e", c=NS, h=4),
        )

        with ExitStack() as s1:
            xa = s1.enter_context(tc.tile_pool(name="xa", bufs=1))
            # wv prefetched on the ACT rail during phase 1a; its pool sits
            # below wqk on the stack so the prefetch isn't gated on wqk reuse
            wvp = s1.enter_context(tc.tile_pool(name="wvp", bufs=1))
            wv_sb = wvp.tile([128, ND, 768], IN_DT)

            # ---- Phase 1a: QT = Wq.T @ x, KT = Wk.T @ x (m on partitions)
            with ExitStack() as s1a:
                wqk = s1a.enter_context(tc.tile_pool(name="wqk", bufs=1))
                wq_sb = wqk.tile([128, ND, 256], IN_DT)
                wk_sb = wqk.tile([128, ND, 256], IN_DT)
                for dc2 in range(0, ND, 2):
                    nc.scalar.dma_start(
                        wq_sb[:, dc2 : dc2 + 2, :], wq_pcm[:, dc2 : dc2 + 2, :]
                    )
                for dc4 in range(0, ND, 4):
                    nc.scalar.dma_start(
                        wk_sb[:, dc4 : dc4 + 4, :], wk_pcm[:, dc4 : dc4 + 4, :]
                    )
                xtile = xa.tile([128, ND, S], IN_DT)
                # split across s-blocks and d-chunks so HWDGE queues overlap;
                # first block per-chunk so the first chain starts sooner
                for dc in range(ND):
                    nc.sync.dma_start(
                        xtile[:, dc, 0:512], xT_pcs[:, dc, 0:512]
                    )
                for ib in range(1, NIB):
                    for dc2 in range(0, ND, 2):
                        nc.sync.dma_start(
                            xtile[:, dc2 : dc2 + 2, ib * 512 : (ib + 1) * 512],
                            xT_pcs[:, dc2 : dc2 + 2, ib * 512 : (ib + 1) * 512],
                        )
                for dc2 in range(0, ND, 2):
                    nc.scalar.dma_start(
                        wv_sb[:, dc2 : dc2 + 2, :], wv_pce[:, dc2 : dc2 + 2, :]
                    )
                pnd = 3 if ABLATE == "noproj" else ND
                for ib in range(NIB):
                    for w_sb, is_q in ((wq_sb, True), (wk_sb, False)):
                        for m2 in range(2):
                            ps = p_proj.tile([128, 512], F32, tag="pmix")
                            for dc in range(pnd):
                                nc.tensor.matmul(
                                    ps[:],
                                    w_sb[:, dc, m2 * 128 : (m2 + 1) * 128],
                                    xtile[:, dc, ib * 512 : (ib + 1) * 512],
                                    start=(dc == 0),
                                    stop=(dc == pnd - 1),
                                )
                            blk = slice(ib * 512, (ib + 1) * 512)
                            if is_q:
                                nc.vector.tensor_copy(qt[:, m2, blk], ps[:])
                            else:
                                nc.vector.tensor_copy(
                                    ktp[0:64, 0, m2, blk], ps[0:64, :]
                                )
                                nc.vector.tensor_copy(
                                    ktp[64:128, 1, m2, blk], ps[64:128, :]
                                )

            # ---- Phase 2 setup: the first two blocks' scores are emitted
            # before the V projection so their ACT exp stream hides entirely
            # under phase 1b's PE work; the rest runs as a lag-1 software
            # pipeline (AV of block n after block n+1's scores) so the PE
            # never waits on the exp stream.
            ep = s1.enter_context(tc.tile_pool(name="ep", bufs=2))

            def emit_scores(pair, ib):
                i0 = ib * 512
                # E holds exp(scores^T/8) for both heads of the pair:
                # head A in [:, jc, 0:512], head B in [:, jc, 512:1024]
                e_sb = ep.tile([128, NS, 1024], e_dt, tag="e")
                mv = 64 if ABLATE == "scores" else 512
                for jc in range(NS):
                    j0 = jc * 128
                    pss = p_sc.tile([128, 1024], F32, tag="pss")
                    nc.tensor.matmul(
                        pss[:, 0:mv],
                        ktp[:, 0, pair, j0 : j0 + 128],
                        qt[:, pair, i0 : i0 + mv],
                        start=True,
                        stop=True,
                    )
                    nc.tensor.matmul(
                        pss[:, 512 : 512 + mv],
                        ktp[:, 1, pair, j0 : j0 + 128],
                        qt[:, pair, i0 : i0 + mv],
                        start=True,
                        stop=True,
                    )
                    if ABLATE == "exp":
                        nc.scalar.activation(
                            e_sb[:, jc, 0:64], pss[:, 0:64], AF.Exp, scale=0.125
                        )
                    else:
                        nc.scalar.activation(
                            e_sb[:, jc, :], pss[:], AF.Exp, scale=0.125
                        )
                return e_sb

            def emit_av(pair, ib, e_sb):
                i0 = ib * 512
                njc = 1 if ABLATE == "noav" else NS
                for hh in range(2):
                    h = pair * 2 + hh
                    for isub in range(4):
                        pav = p_av.tile([128, dvp], F32, tag="pmix")
                        for jc in range(njc):
                            nc.tensor.matmul(
                                pav[:],
                                e_sb[
                                    :,
                                    jc,
                                    hh * 512 + isub * 128 : hh * 512
                                    + (isub + 1) * 128,
                                ],
                                vp[:, jc, h, :],
                                start=(jc == 0),
                                stop=(jc == njc - 1),
                            )
                        rec = mp.tile([128, 1], F32, tag="rec")
                        nc.vector.reciprocal(rec[:], pav[:, DV : DV + 1])
                        ot = mp.tile([128, DV], F32, tag="ot")
                        nc.vector.tensor_scalar_mul(ot[:], pav[:, 0:DV], rec[:])
                        r0 = i0 + isub * 128
                        nc.sync.dma_start(out_ap[h, r0 : r0 + 128, :], ot[:])

            blocks = [(pair, ib) for pair in range(2) for ib in range(NIB)]
            pending = []
            for pair, ib in blocks[:2]:
                pending.append((pair, ib, emit_scores(pair, ib)))

            # ---- Phase 1b: V = x @ Wv (natural layout: s on partitions)
            vnd = 3 if ABLATE == "noproj" else ND
            for sc in range(NS):
                c0 = sc * 128
                for e2 in range(2):
                    ps = p_proj.tile([128, 384], F32, tag="pmix")
                    for dc in range(vnd):
                        nc.tensor.matmul(
                            ps[:],
                            xtile[:, dc, c0 : c0 + 128],
                            wv_sb[:, dc, e2 * 384 : (e2 + 1) * 384],
                            start=(dc == 0),
                            stop=(dc == vnd - 1),
                        )
                    nc.vector.tensor_copy(vp[:, sc, 2 * e2, 0:DV], ps[:, 0:DV])
                    nc.vector.tensor_copy(
                        vp[:, sc, 2 * e2 + 1, 0:DV], ps[:, DV : 2 * DV]
                    )

            # ---- Phase 2 tail
            for pair, ib in blocks[2:]:
                emit_av(*pending.pop(0))
                pending.append((pair, ib, emit_scores(pair, ib)))
            for blk in pending:
                emit_av(*blk)


def make_vpad() -> np.ndarray:
    import ml_dtypes

    dvp = (DV + 1) if AV_BF16 else 256
    dt = ml_dtypes.bfloat16 if AV_BF16 else np.float32
    pad = np.zeros((128, NS, 4, dvp - DV), dt)
    pad[:, :, :, 0] = 1.0
    return pad.reshape(128, -1)


def shard_inputs(inputs, Wq, Wk, Wv):
    vpad = make_vpad()
    import ml_dtypes

    bf = ml_dtypes.bfloat16
    in_maps = []
    for c in range(8):
        b, g = c // 2, c % 2
        in_maps.append(
            {
                "xT": np.asarray(inputs[b]).T.astype(bf),
                "wq": np.asarray(Wq[:, g * 256 : (g + 1) * 256]).astype(bf),
                "wk": np.asarray(Wk[:, g * 256 : (g + 1) * 256]).astype(bf),
                "wv": np.asarray(Wv[:, g * 768 : (g + 1) * 768]).astype(bf),
                "vpad": vpad,
            }
        )
    return in_maps


def gather_outputs(results):
    full = np.empty((B, S, 1536), np.float32)
    for c, r in enumerate(results):
        b, g = c // 2, c % 2
        o = r["out"]
        for h in range(4):
            full[b, :, g * 768 + h * DV : g * 768 + (h + 1) * DV] = o[h]
    return full


_cached = {}


def kernel(inputs, Wq, Wk, Wv) -> np.ndarray:
    """Full [4,1536,1536] fp32 MHA forward, computed on 8 NeuronCores."""
    _install_waitsplit()
    inputs = np.asarray(inputs, dtype=np.float32)
    Wq = np.asarray(Wq, dtype=np.float32)
    Wk = np.asarray(Wk, dtype=np.float32)
    Wv = np.asarray(Wv, dtype=np.float32)

    if "nc" not in _cached:
        _cached["nc"] = build_kernel()
    nc = _cached["nc"]
    in_maps = shard_inputs(inputs, Wq, Wk, Wv)

    last_err = None
    for _attempt in range(3):
        try:
            res = run_bass_kernel_spmd(nc, in_maps, core_ids=list(range(8)))
            return gather_outputs(res.results)
        except Exception as e:  # wedged-device retry
            last_err = e
    raise last_err

